# revision 21
# baseline (speedup 1.0000x reference)
"""DeltaRule (diagonal-state linear attention) Bass kernel for 8 TRN2 cores.

Problem: nn_DeltaRule_20194936225992
  B=4, S=2048, H_DIM=1024, N_HEADS=16, HEAD_DIM=64.
  q/k/v/b projections, phi = elu+1, per-(b,h,d) scalar linear recurrence
      s_t = (1 - b_t*pk_t^2) * s_{t-1} + b_t*v_t*pk_t ;  y_t = s_t * pq_t
  out = y @ Wo.T + bo

Sharding: core = (batch b, head-group hg) with hg covering 8 heads.
Each core computes its partial O-projection (contraction over its 512
lanes); host sums the two head-group partials per batch, transposes
[o,t] -> [t,o] and adds bo.

On-device layout: lanes (h*64+d) on partitions, time on free dim.  The
recurrence runs as a hardware `tensor_tensor_scan` per [128,TC] tile,
chained across time chunks via the last column of the previous s.

Engine plan (per lane-tile, per chunk):
  PE:  Wq/Wk/Wv projections (weights stationary, x.T moving), O-projection
       of the PREVIOUS chunk (software-pipelined between this chunk's v and
       q phases so the PE never waits on the elementwise chain).
  ACT: E = exp(x+b) for phi (one op per projection via the exact identity
       phi(x) = elu(x)+1 = min(exp(x), 1+relu(x))), PSUM->SBUF O copies
       with fp32->fp16 downcast.
  DVE: u = max(x+b+1, 1), pk = min(u, E), a = 1-g, cc = v*w, the scan,
       q-side u/pq, y = s*pq.  fp16 intermediates use the DVE 2x/4x modes.
  GpSimd: w = pk*b, g = pk*w (idle engine; off the critical rate path).
  The sigmoid gate b is computed on the host (0.4% of total FLOPs) and
  DMA'd pre-broadcast per lane.

All tensors fp16 on device except PSUM accumulators, the fp32 scan state
(internal to the scan instruction), and the per-lane bias columns.
Chunk-0 weight/x DMAs are sliced into 2-dt pieces and interleaved so the
first matmul issues at ~2us instead of ~16us.  The v-projection bias
matmul is skipped when bv == 0 (it is for this problem's inputs).
"""

import os
import sys

for _p in ("/opt/trn_rl_repo", os.path.expanduser("~/.axon_site/_ro/trn_rl_repo")):
    if os.path.isdir(_p) and _p not in sys.path:
        sys.path.insert(0, _p)

import numpy as np  # noqa: E402

import concourse.bass as bass  # noqa: E402
import concourse.tile as tile  # noqa: E402
from concourse import bacc, mybir  # noqa: E402
from concourse.bass import ts  # noqa: E402
from concourse.bass_utils import run_bass_kernel_spmd  # noqa: E402

# problem constants (hardcoded per task rules)
B, S, H_DIM, N_HEADS, HEAD_DIM = 4, 2048, 1024, 16, 64
P = 128
NCORES = 8
HG = 2                      # head groups
J = 512                     # lanes per core  (8 heads * 64)
JT = J // P                 # 4 j-tiles
DT = H_DIM // P             # 8 contraction tiles
HPC = N_HEADS // HG         # 8 heads per core
TC = 512                    # time chunk (max PE moving dim / PSUM bank)
NCH = S // TC
# chunk-0 DMA piece plan (dt_start, n_dt): finer first pieces so the first
# matmul issues as early as possible
PLAN = [(0, 1), (1, 1), (2, 2), (4, 2), (6, 2)]
NPC = len(PLAN)
DT2PIECE = {}
for _i, (_s, _n) in enumerate(PLAN):
    for _d in range(_s, _s + _n):
        DT2PIECE[_d] = (_i, _d - _s)

F32 = mybir.dt.float32
F16 = mybir.dt.float16
AF = mybir.ActivationFunctionType
M = mybir.AluOpType

# env knobs for experiments
GP_WG = os.environ.get("DELTA_GP_WG", "0") != "0"   # w,g on GpSimd (slower)
PIPE_O = os.environ.get("DELTA_PIPE", "1") != "0"   # software-pipeline O-proj
N_WARM = int(os.environ.get("DELTA_WARM", "8"))     # PE p-state warmup matmuls


def build_nc(with_vbias):
    nc = bacc.Bacc(trn_type="TRN2", target_bir_lowering=False, debug=False)

    # per-core inputs; x tensors host-packed as [p, dt, s] fp16
    xq = nc.dram_tensor("xq", [P, DT, S], F16, kind="ExternalInput").ap()
    xk = nc.dram_tensor("xk", [P, DT, S], F16, kind="ExternalInput").ap()
    xv = nc.dram_tensor("xv", [P, DT, S], F16, kind="ExternalInput").ap()
    bbb = nc.dram_tensor("bbb", [P, JT, S], F16, kind="ExternalInput").ap()
    wq = nc.dram_tensor("wq", [H_DIM, J], F16, kind="ExternalInput").ap()
    wk = nc.dram_tensor("wk", [H_DIM, J], F16, kind="ExternalInput").ap()
    wv = nc.dram_tensor("wv", [H_DIM, J], F16, kind="ExternalInput").ap()
    wo = nc.dram_tensor("wo", [J, H_DIM], F16, kind="ExternalInput").ap()
    bq = nc.dram_tensor("bq", [P, JT], F32, kind="ExternalInput").ap()
    bk = nc.dram_tensor("bk", [P, JT], F32, kind="ExternalInput").ap()
    bq1 = nc.dram_tensor("bq1", [P, JT], F32, kind="ExternalInput").ap()
    bk1 = nc.dram_tensor("bk1", [P, JT], F32, kind="ExternalInput").ap()
    if with_vbias:
        bvr = nc.dram_tensor("bvr", [1, J], F16, kind="ExternalInput").ap()
    out = nc.dram_tensor("out", [H_DIM, S], F16, kind="ExternalOutput").ap()

    from contextlib import ExitStack

    with tile.TileContext(nc) as tcx, ExitStack() as ctx:
        wpool = ctx.enter_context(tcx.tile_pool(name="weights", bufs=1))
        c0pool = ctx.enter_context(tcx.tile_pool(name="c0x", bufs=1))
        xpool = ctx.enter_context(tcx.tile_pool(name="xin", bufs=2))
        ipool = ctx.enter_context(tcx.tile_pool(name="inter", bufs=3))
        spool = ctx.enter_context(tcx.tile_pool(name="scan", bufs=2))
        opool = ctx.enter_context(tcx.tile_pool(name="osb", bufs=4))
        pproj = ctx.enter_context(tcx.tile_pool(name="pproj", bufs=5, space="PSUM"))
        po = ctx.enter_context(tcx.tile_pool(name="po", bufs=3, space="PSUM"))

        # --- PE p-state warmup: K=1 matmuls on zeroed operands keep the PE
        # continuously busy through the DMA-bound preamble so the first real
        # matmul runs at the full 2.4GHz clock instead of ramping ---
        if N_WARM:
            wa = wpool.tile([1, P], F16, tag="warm_a")
            wb = wpool.tile([1, TC], F16, tag="warm_b")
            nc.vector.memset(wa[:], 0.0)
            nc.vector.memset(wb[:], 0.0)
            pwarm = po.tile([P, TC], F32, tag="po", name="pwarm")
            for _ in range(N_WARM):
                nc.tensor.matmul(out=pwarm[:], lhsT=wa[:], rhs=wb[:],
                                 start=True, stop=True)

        # --- persistent weights / constants (per-2dt pieces) ---
        def wpieces(tag):
            return [wpool.tile([P, n, J], F16, tag=f"{tag}{i}", name=f"{tag}{i}")
                    for i, (_, n) in enumerate(PLAN)]

        wk_p, wv_p, wq_p = wpieces("wk"), wpieces("wv"), wpieces("wq")
        wo_sb = wpool.tile([P, JT, H_DIM], F16, tag="wo")
        bq_sb = wpool.tile([P, JT], F32, tag="bq")
        bk_sb = wpool.tile([P, JT], F32, tag="bk")
        bq1_sb = wpool.tile([P, JT], F32, tag="bq1")
        bk1_sb = wpool.tile([P, JT], F32, tag="bk1")
        if with_vbias:
            bvr_sb = wpool.tile([1, J], F16, tag="bvr")
            ones_sb = wpool.tile([1, TC], F16, tag="ones")

        # chunk-0 x pieces (separate tiles so matmuls start per-piece)
        def xpieces(tag):
            return [c0pool.tile([P, n, TC], F16, tag=f"{tag}{i}", name=f"{tag}{i}")
                    for i, (_, n) in enumerate(PLAN)]

        xk0_p, xv0_p, xq0_p = xpieces("xk0"), xpieces("xv0"), xpieces("xq0")

        def wsrc(w_p, d, jsl):
            i, off = DT2PIECE[d]
            return w_p[i][:, off, jsl]

        # --- chunk-0 startup DMA stream: interleave weights with x pieces ---
        def wdma(w_p, wt, i):
            s, n = PLAN[i]
            nc.sync.dma_start(
                out=w_p[i][:],
                in_=wt.rearrange("(dt p) j -> p dt j", p=P)[:, s:s + n, :])

        def xdma(x_p, xt, i):
            s, n = PLAN[i]
            nc.sync.dma_start(out=x_p[i][:], in_=xt[:, s:s + n, 0:TC])

        for i in range(NPC):
            wdma(wk_p, wk, i)
            xdma(xk0_p, xk, i)
            if i == 0:
                nc.sync.dma_start(out=bk_sb[:], in_=bk)
                nc.sync.dma_start(out=bk1_sb[:], in_=bk1)
        for i in range(NPC):
            wdma(wv_p, wv, i)
            xdma(xv0_p, xv, i)
        bb_c = xpool.tile([P, JT, TC], F16, tag="bbb")
        nc.sync.dma_start(out=bb_c[:], in_=bbb[:, :, 0:TC])
        if with_vbias:
            nc.sync.dma_start(out=bvr_sb[:], in_=bvr)
            nc.vector.memset(ones_sb[:], 1.0)
        for i in range(NPC):
            wdma(wq_p, wq, i)
            xdma(xq0_p, xq, i)
        nc.sync.dma_start(out=bq_sb[:], in_=bq)
        nc.sync.dma_start(out=bq1_sb[:], in_=bq1)
        # wo is dispatched after chunk-1's x prefetch (first needed by the
        # pipelined O(0) midway through chunk 1) to keep it out of the
        # DMA-backlog critical path

        s_prev = [None] * JT    # last-chunk scan state tile per lane-tile
        y_prev = [None] * JT    # previous chunk's y tiles (for pipelined O)

        eng_wg = nc.gpsimd if GP_WG else nc.vector

        def emit_O(cp, y_tiles, final=False):
            """O-projection of chunk cp: out[o,t] += wo[j,o]*y[j,t].

            The final chunk's PSUM->SBUF copies go to DVE (idle in the tail,
            while the ACT queue still drains the q-phase Exp ops)."""
            for ot in range(DT):
                pso = po.tile([P, TC], F32, tag="po")
                for lt in range(JT):
                    nc.tensor.matmul(
                        out=pso[:], lhsT=wo_sb[:, lt, ts(ot, P)],
                        rhs=y_tiles[lt][:],
                        start=(lt == 0), stop=(lt == JT - 1),
                    )
                o_sb = opool.tile([P, TC], F16, tag="osb")
                if final:
                    nc.vector.tensor_copy(out=o_sb[:], in_=pso[:])
                else:
                    nc.scalar.copy(out=o_sb[:], in_=pso[:])
                nc.sync.dma_start(out=out[ts(ot, P), ts(cp, TC)], in_=o_sb[:])

        for c in range(NCH):
            # --- stream x chunk (c>0); chunk 0 was sliced above ---
            if c > 0:
                xk_c = xpool.tile([P, DT, TC], F16, tag="xk")
                nc.sync.dma_start(out=xk_c[:], in_=xk[:, :, ts(c, TC)])
                xv_c = xpool.tile([P, DT, TC], F16, tag="xv")
                nc.sync.dma_start(out=xv_c[:], in_=xv[:, :, ts(c, TC)])
                bb_c = xpool.tile([P, JT, TC], F16, tag="bbb")
                nc.sync.dma_start(out=bb_c[:], in_=bbb[:, :, ts(c, TC)])
                xq_c = xpool.tile([P, DT, TC], F16, tag="xq")
                nc.sync.dma_start(out=xq_c[:], in_=xq[:, :, ts(c, TC)])
                if c == 1:
                    nc.sync.dma_start(
                        out=wo_sb[:],
                        in_=wo.rearrange("(jt p) o -> p jt o", p=P))

            def xsrc(whole, pieces, d):
                if c == 0:
                    i, off = DT2PIECE[d]
                    return pieces[i][:, off, :]
                return whole[:, d, :]

            # ---- k projections + phi(k) + scan coefficients ----
            pk_t, w_t = [], []
            for lt in range(JT):
                jsl = ts(lt, P)
                psk = pproj.tile([P, TC], F32, tag="proj")
                for d in range(DT):
                    nc.tensor.matmul(
                        out=psk[:], lhsT=wsrc(wk_p, d, jsl),
                        rhs=xsrc(xk_c if c else None, xk0_p, d),
                        start=(d == 0), stop=(d == DT - 1),
                    )
                # phi(x) = min(exp(x), 1 + relu(x)), exact
                ek = ipool.tile([P, TC], F16, tag="ek")
                nc.scalar.activation(out=ek[:], in_=psk[:], func=AF.Exp,
                                     bias=bk_sb[:, lt:lt + 1])
                uk = ipool.tile([P, TC], F16, tag="uk")
                nc.vector.tensor_scalar(
                    out=uk[:], in0=psk[:], scalar1=bk1_sb[:, lt:lt + 1],
                    scalar2=1.0, op0=M.add, op1=M.max)
                pk = ipool.tile([P, TC], F16, tag="pk")
                nc.vector.tensor_tensor(out=pk[:], in0=uk[:], in1=ek[:], op=M.min)
                pk_t.append(pk)
                w = ipool.tile([P, TC], F16, tag="w")
                eng_wg.tensor_tensor(out=w[:], in0=pk[:], in1=bb_c[:, lt, :],
                                     op=M.mult)
                w_t.append(w)

            # ---- previous chunk's O-projection (fills the PE while this
            # chunk's elementwise chain completes; placed after the k phase
            # so its PSUM->SBUF copies clear the ACT queue before eq) ----
            if PIPE_O and c > 0:
                emit_O(c - 1, y_prev)

            # ---- v projections + scan ----
            s_new_t = []
            for lt in range(JT):
                jsl = ts(lt, P)
                psv = pproj.tile([P, TC], F32, tag="proj")
                for d in range(DT):
                    nc.tensor.matmul(
                        out=psv[:], lhsT=wsrc(wv_p, d, jsl),
                        rhs=xsrc(xv_c if c else None, xv0_p, d),
                        start=(d == 0), stop=(d == DT - 1) and not with_vbias,
                    )
                if with_vbias:
                    nc.tensor.matmul(out=psv[:], lhsT=bvr_sb[:, jsl],
                                     rhs=ones_sb[:], start=False, stop=True)
                pk, w = pk_t[lt], w_t[lt]
                g = ipool.tile([P, TC], F16, tag="g")
                eng_wg.tensor_tensor(out=g[:], in0=pk[:], in1=w[:], op=M.mult)
                a = ipool.tile([P, TC], F16, tag="a")
                nc.vector.tensor_scalar(out=a[:], in0=g[:], scalar1=-1.0,
                                        scalar2=1.0, op0=M.mult, op1=M.add)
                cc = ipool.tile([P, TC], F16, tag="cc")
                nc.vector.tensor_tensor(out=cc[:], in0=psv[:], in1=w[:], op=M.mult)
                s_new = spool.tile([P, TC], F16, tag=f"s{lt}")
                init = 0.0 if c == 0 else s_prev[lt][:, TC - 1:TC]
                nc.vector.tensor_tensor_scan(
                    out=s_new[:], data0=a[:], data1=cc[:], initial=init,
                    op0=M.mult, op1=M.add,
                )
                s_prev[lt] = s_new
                s_new_t.append(s_new)

            # ---- q projections + phi(q) + y = s * pq ----
            y_new = []
            for lt in range(JT):
                jsl = ts(lt, P)
                psq = pproj.tile([P, TC], F32, tag="proj")
                for d in range(DT):
                    nc.tensor.matmul(
                        out=psq[:], lhsT=wsrc(wq_p, d, jsl),
                        rhs=xsrc(xq_c if c else None, xq0_p, d),
                        start=(d == 0), stop=(d == DT - 1),
                    )
                eq = ipool.tile([P, TC], F16, tag="ek")
                nc.scalar.activation(out=eq[:], in_=psq[:], func=AF.Exp,
                                     bias=bq_sb[:, lt:lt + 1])
                uq = ipool.tile([P, TC], F16, tag="uk")
                nc.vector.tensor_scalar(
                    out=uq[:], in0=psq[:], scalar1=bq1_sb[:, lt:lt + 1],
                    scalar2=1.0, op0=M.add, op1=M.max)
                pq = ipool.tile([P, TC], F16, tag="pk")
                nc.vector.tensor_tensor(out=pq[:], in0=uq[:], in1=eq[:], op=M.min)
                y = spool.tile([P, TC], F16, tag=f"y{lt}")
                nc.vector.tensor_tensor(out=y[:], in0=s_new_t[lt][:], in1=pq[:],
                                        op=M.mult)
                y_new.append(y)
            y_prev = y_new

            if not PIPE_O:
                emit_O(c, y_prev)

        if PIPE_O:
            emit_O(NCH - 1, y_prev, final=True)

    nc.compile()
    return nc


_NC_CACHE = {}


def _get_nc(with_vbias):
    key = (with_vbias, GP_WG, PIPE_O)
    if key not in _NC_CACHE:
        _NC_CACHE[key] = build_nc(with_vbias)
    return _NC_CACHE[key]


def make_in_maps(query, key, value, beta, Wq, bq, Wk, bk, Wv, bv, Wb, bb, Wo, bo,
                 with_vbias):
    """Host-side shard prep: core_id = b*2 + hg."""

    def xpack(x):  # [S, H_DIM] -> [p, dt, s] fp16
        a = np.asarray(x, np.float32).T.reshape(DT, P, S)
        return np.ascontiguousarray(a.transpose(1, 0, 2)).astype(np.float16)

    def t16(x):
        return np.ascontiguousarray(np.asarray(x, np.float32).T).astype(np.float16)

    xqs = [xpack(query[b]) for b in range(B)]
    xks = [xpack(key[b]) for b in range(B)]
    xvs = [xpack(value[b]) for b in range(B)]
    # gate b computed host-side (0.4% of FLOPs), pre-broadcast per lane
    Wbf = np.asarray(Wb, np.float32)
    bbf0 = np.asarray(bb, np.float32)
    z = np.einsum('bsd,hd->bsh', np.asarray(beta, np.float32), Wbf) + bbf0
    bgate = 1.0 / (1.0 + np.exp(-z))                      # [B, S, 16]

    bqf = np.asarray(bq, np.float32)
    bkf = np.asarray(bk, np.float32)
    bvf = np.asarray(bv, np.float32)

    in_maps = []
    for b in range(B):
        for hg in range(HG):
            jsl = slice(hg * J, (hg + 1) * J)
            hsl = slice(hg * HPC, (hg + 1) * HPC)

            def lanes(v):  # [J] -> [128, 4] per lane-tile columns
                return np.ascontiguousarray(v[jsl].reshape(JT, P).T)

            # [S, 512] lane-broadcast gate -> [p, lt, s]
            rep = np.repeat(bgate[b][:, hsl], HEAD_DIM, axis=1).T  # [512, S]
            bl = np.ascontiguousarray(
                rep.reshape(JT, P, S).transpose(1, 0, 2)).astype(np.float16)

            m = {
                "xq": xqs[b], "xk": xks[b], "xv": xvs[b], "bbb": bl,
                "wq": t16(Wq[jsl]), "wk": t16(Wk[jsl]), "wv": t16(Wv[jsl]),
                "wo": t16(Wo[:, jsl]),
                "bq": lanes(bqf), "bk": lanes(bkf),
                "bq1": lanes(bqf) + 1.0, "bk1": lanes(bkf) + 1.0,
            }
            if with_vbias:
                m["bvr"] = bvf[jsl].reshape(1, J).astype(np.float16)
            in_maps.append(m)
    return in_maps


LAST_RESULTS = None


def kernel(**inputs):
    global LAST_RESULTS
    with_vbias = bool(np.any(np.asarray(inputs["bv"], np.float32)))
    nc = _get_nc(with_vbias)
    in_maps = make_in_maps(**inputs, with_vbias=with_vbias)
    res = run_bass_kernel_spmd(nc, in_maps, core_ids=list(range(NCORES)),
                               trace=bool(os.environ.get("DELTA_TRACE")))
    LAST_RESULTS = res
    bo = np.asarray(inputs["bo"], np.float32)
    out = np.empty((B, S, H_DIM), np.float32)
    for b in range(B):
        m = (res.results[2 * b]["out"].astype(np.float32)
             + res.results[2 * b + 1]["out"].astype(np.float32))
        out[b] = m.T + bo
    return out


# revision 26
# speedup vs baseline: 1.0394x; 1.0394x over previous
"""DeltaRule (diagonal-state linear attention) Bass kernel for 8 TRN2 cores.

Problem: nn_DeltaRule_20194936225992
  B=4, S=2048, H_DIM=1024, N_HEADS=16, HEAD_DIM=64.
  q/k/v/b projections, phi = elu+1, per-(b,h,d) scalar linear recurrence
      s_t = (1 - b_t*pk_t^2) * s_{t-1} + b_t*v_t*pk_t ;  y_t = s_t * pq_t
  out = y @ Wo.T + bo

Sharding: core = (batch b, head-group hg) with hg covering 8 heads.
Each core computes its partial O-projection (contraction over its 512
lanes); host sums the two head-group partials per batch, transposes
[o,t] -> [t,o] and adds bo.

On-device layout: lanes (h*64+d) on partitions, time on free dim.  The
recurrence runs as a hardware `tensor_tensor_scan` per [128,TC] tile,
chained across time chunks via the last column of the previous s.

Engine plan (per lane-tile, per chunk):
  PE:  Wq/Wk/Wv projections (weights stationary, x.T moving), O-projection
       of the PREVIOUS chunk (software-pipelined between this chunk's v and
       q phases so the PE never waits on the elementwise chain).
  ACT: E = exp(x+b) for phi (one op per projection via the exact identity
       phi(x) = elu(x)+1 = min(exp(x), 1+relu(x))), PSUM->SBUF O copies
       with fp32->fp16 downcast.
  DVE: u = max(x+b+1, 1), pk = min(u, E), a = 1-g, cc = v*w, the scan,
       q-side u/pq, y = s*pq.  fp16 intermediates use the DVE 2x/4x modes.
  GpSimd: w = pk*b, g = pk*w (idle engine; off the critical rate path).
  The sigmoid gate b is computed on the host (0.4% of total FLOPs) and
  DMA'd pre-broadcast per lane.

All tensors fp16 on device except PSUM accumulators, the fp32 scan state
(internal to the scan instruction), and the per-lane bias columns.
Chunk-0 weight/x DMAs are sliced into 2-dt pieces and interleaved so the
first matmul issues at ~2us instead of ~16us.  The v-projection bias
matmul is skipped when bv == 0 (it is for this problem's inputs).
"""

import os
import sys

for _p in ("/opt/trn_rl_repo", os.path.expanduser("~/.axon_site/_ro/trn_rl_repo")):
    if os.path.isdir(_p) and _p not in sys.path:
        sys.path.insert(0, _p)

import numpy as np  # noqa: E402

import concourse.bass as bass  # noqa: E402
import concourse.tile as tile  # noqa: E402
from concourse import bacc, mybir  # noqa: E402
from concourse.bass import ts  # noqa: E402
from concourse.bass_utils import run_bass_kernel_spmd  # noqa: E402

# problem constants (hardcoded per task rules)
B, S, H_DIM, N_HEADS, HEAD_DIM = 4, 2048, 1024, 16, 64
P = 128
NCORES = 8
HG = 2                      # head groups
J = 512                     # lanes per core  (8 heads * 64)
JT = J // P                 # 4 j-tiles
DT = H_DIM // P             # 8 contraction tiles
HPC = N_HEADS // HG         # 8 heads per core
TC = 512                    # time chunk (max PE moving dim / PSUM bank)
NCH = S // TC
# chunk-0 DMA piece plan (dt_start, n_dt): sliced so the first matmul can
# issue after one piece instead of the whole 1MB tensor
PLAN = [(0, 2), (2, 2), (4, 2), (6, 2)]
NPC = len(PLAN)
DT2PIECE = {}
for _i, (_s, _n) in enumerate(PLAN):
    for _d in range(_s, _s + _n):
        DT2PIECE[_d] = (_i, _d - _s)

F32 = mybir.dt.float32
F16 = mybir.dt.float16
AF = mybir.ActivationFunctionType
M = mybir.AluOpType

# env knobs for experiments
GP_WG = os.environ.get("DELTA_GP_WG", "0") != "0"   # w,g on GpSimd (slower)
PIPE_O = os.environ.get("DELTA_PIPE", "1") != "0"   # software-pipeline O-proj
N_WARM = int(os.environ.get("DELTA_WARM", "8"))     # PE p-state warmup matmuls
PP_BUFS = int(os.environ.get("DELTA_PP", "5"))      # pproj PSUM banks
PO_BUFS = int(os.environ.get("DELTA_PO", "3"))      # O-proj PSUM banks


def build_nc(with_vbias):
    nc = bacc.Bacc(trn_type="TRN2", target_bir_lowering=False, debug=False)

    # per-core inputs; x tensors host-packed as [p, dt, s] fp16
    xq = nc.dram_tensor("xq", [P, DT, S], F16, kind="ExternalInput").ap()
    xk = nc.dram_tensor("xk", [P, DT, S], F16, kind="ExternalInput").ap()
    xv = nc.dram_tensor("xv", [P, DT, S], F16, kind="ExternalInput").ap()
    bbb = nc.dram_tensor("bbb", [P, JT, S], F16, kind="ExternalInput").ap()
    wq = nc.dram_tensor("wq", [H_DIM, J], F16, kind="ExternalInput").ap()
    wk = nc.dram_tensor("wk", [H_DIM, J], F16, kind="ExternalInput").ap()
    wv = nc.dram_tensor("wv", [H_DIM, J], F16, kind="ExternalInput").ap()
    wo = nc.dram_tensor("wo", [J, H_DIM], F16, kind="ExternalInput").ap()
    bq = nc.dram_tensor("bq", [P, JT], F32, kind="ExternalInput").ap()
    bk = nc.dram_tensor("bk", [P, JT], F32, kind="ExternalInput").ap()
    bq1 = nc.dram_tensor("bq1", [P, JT], F32, kind="ExternalInput").ap()
    bk1 = nc.dram_tensor("bk1", [P, JT], F32, kind="ExternalInput").ap()
    if with_vbias:
        bvr = nc.dram_tensor("bvr", [1, J], F16, kind="ExternalInput").ap()
    out = nc.dram_tensor("out", [H_DIM, S], F16, kind="ExternalOutput").ap()

    from contextlib import ExitStack

    with tile.TileContext(nc) as tcx, ExitStack() as ctx:
        wpool = ctx.enter_context(tcx.tile_pool(name="weights", bufs=1))
        c0pool = ctx.enter_context(tcx.tile_pool(name="c0x", bufs=1))
        xpool = ctx.enter_context(tcx.tile_pool(name="xin", bufs=2))
        ipool = ctx.enter_context(tcx.tile_pool(name="inter", bufs=3))
        spool = ctx.enter_context(tcx.tile_pool(name="scan", bufs=2))
        opool = ctx.enter_context(tcx.tile_pool(name="osb", bufs=4))
        pproj = ctx.enter_context(
            tcx.tile_pool(name="pproj", bufs=PP_BUFS, space="PSUM"))
        po = ctx.enter_context(tcx.tile_pool(name="po", bufs=PO_BUFS, space="PSUM"))

        # --- PE p-state warmup: K=1 matmuls on zeroed operands keep the PE
        # continuously busy through the DMA-bound preamble so the first real
        # matmul runs at the full 2.4GHz clock instead of ramping ---
        if N_WARM:
            wa = wpool.tile([1, P], F16, tag="warm_a")
            wb = wpool.tile([1, TC], F16, tag="warm_b")
            nc.vector.memset(wa[:], 0.0)
            nc.vector.memset(wb[:], 0.0)
            pwarm = po.tile([P, TC], F32, tag="po", name="pwarm")
            for _ in range(N_WARM):
                nc.tensor.matmul(out=pwarm[:], lhsT=wa[:], rhs=wb[:],
                                 start=True, stop=True)

        # --- persistent weights / constants (per-2dt pieces) ---
        def wpieces(tag):
            return [wpool.tile([P, n, J], F16, tag=f"{tag}{i}", name=f"{tag}{i}")
                    for i, (_, n) in enumerate(PLAN)]

        wk_p, wv_p, wq_p = wpieces("wk"), wpieces("wv"), wpieces("wq")
        wo_sb = wpool.tile([P, JT, H_DIM], F16, tag="wo")
        bq_sb = wpool.tile([P, JT], F32, tag="bq")
        bk_sb = wpool.tile([P, JT], F32, tag="bk")
        bq1_sb = wpool.tile([P, JT], F32, tag="bq1")
        bk1_sb = wpool.tile([P, JT], F32, tag="bk1")
        if with_vbias:
            bvr_sb = wpool.tile([1, J], F16, tag="bvr")
            ones_sb = wpool.tile([1, TC], F16, tag="ones")

        # chunk-0 x pieces (separate tiles so matmuls start per-piece)
        def xpieces(tag):
            return [c0pool.tile([P, n, TC], F16, tag=f"{tag}{i}", name=f"{tag}{i}")
                    for i, (_, n) in enumerate(PLAN)]

        xk0_p, xv0_p, xq0_p = xpieces("xk0"), xpieces("xv0"), xpieces("xq0")

        def wsrc(w_p, d, jsl):
            i, off = DT2PIECE[d]
            return w_p[i][:, off, jsl]

        # --- chunk-0 startup DMA stream: interleave weights with x pieces ---
        def wdma(w_p, wt, i):
            s, n = PLAN[i]
            nc.sync.dma_start(
                out=w_p[i][:],
                in_=wt.rearrange("(dt p) j -> p dt j", p=P)[:, s:s + n, :])

        def xdma(x_p, xt, i):
            s, n = PLAN[i]
            nc.sync.dma_start(out=x_p[i][:], in_=xt[:, s:s + n, 0:TC])

        for i in range(NPC):
            wdma(wk_p, wk, i)
            xdma(xk0_p, xk, i)
            if i == 0:
                nc.sync.dma_start(out=bk_sb[:], in_=bk)
                nc.sync.dma_start(out=bk1_sb[:], in_=bk1)
        for i in range(NPC):
            wdma(wv_p, wv, i)
            xdma(xv0_p, xv, i)
        bb_c = xpool.tile([P, JT, TC], F16, tag="bbb")
        nc.sync.dma_start(out=bb_c[:], in_=bbb[:, :, 0:TC])
        if with_vbias:
            nc.sync.dma_start(out=bvr_sb[:], in_=bvr)
            nc.vector.memset(ones_sb[:], 1.0)
        for i in range(NPC):
            wdma(wq_p, wq, i)
            xdma(xq0_p, xq, i)
        nc.sync.dma_start(out=bq_sb[:], in_=bq)
        nc.sync.dma_start(out=bq1_sb[:], in_=bq1)
        # wo is dispatched after chunk-1's x prefetch (first needed by the
        # pipelined O(0) midway through chunk 1) to keep it out of the
        # DMA-backlog critical path

        s_prev = [None] * JT    # last-chunk scan state tile per lane-tile
        y_prev = [None] * JT    # previous chunk's y tiles (for pipelined O)

        eng_wg = nc.gpsimd if GP_WG else nc.vector

        def emit_O(cp, y_tiles, final=False):
            """O-projection of chunk cp: out[o,t] += wo[j,o]*y[j,t].

            The final chunk's PSUM->SBUF copies go to DVE (idle in the tail,
            while the ACT queue still drains the q-phase Exp ops)."""
            for ot in range(DT):
                pso = po.tile([P, TC], F32, tag="po")
                for lt in range(JT):
                    nc.tensor.matmul(
                        out=pso[:], lhsT=wo_sb[:, lt, ts(ot, P)],
                        rhs=y_tiles[lt][:],
                        start=(lt == 0), stop=(lt == JT - 1),
                    )
                o_sb = opool.tile([P, TC], F16, tag="osb")
                nc.scalar.copy(out=o_sb[:], in_=pso[:])
                nc.sync.dma_start(out=out[ts(ot, P), ts(cp, TC)], in_=o_sb[:])

        for c in range(NCH):
            # --- stream x chunk (c>0); chunk 0 was sliced above ---
            if c > 0:
                xk_c = xpool.tile([P, DT, TC], F16, tag="xk")
                nc.sync.dma_start(out=xk_c[:], in_=xk[:, :, ts(c, TC)])
                xv_c = xpool.tile([P, DT, TC], F16, tag="xv")
                nc.sync.dma_start(out=xv_c[:], in_=xv[:, :, ts(c, TC)])
                bb_c = xpool.tile([P, JT, TC], F16, tag="bbb")
                nc.sync.dma_start(out=bb_c[:], in_=bbb[:, :, ts(c, TC)])
                xq_c = xpool.tile([P, DT, TC], F16, tag="xq")
                nc.sync.dma_start(out=xq_c[:], in_=xq[:, :, ts(c, TC)])
                if c == 1:
                    nc.sync.dma_start(
                        out=wo_sb[:],
                        in_=wo.rearrange("(jt p) o -> p jt o", p=P))

            def xsrc(whole, pieces, d):
                if c == 0:
                    i, off = DT2PIECE[d]
                    return pieces[i][:, off, :]
                return whole[:, d, :]

            # ---- k projections + phi(k) + scan coefficients ----
            pk_t, w_t = [], []
            for lt in range(JT):
                jsl = ts(lt, P)
                psk = pproj.tile([P, TC], F32, tag="proj")
                for d in range(DT):
                    nc.tensor.matmul(
                        out=psk[:], lhsT=wsrc(wk_p, d, jsl),
                        rhs=xsrc(xk_c if c else None, xk0_p, d),
                        start=(d == 0), stop=(d == DT - 1),
                    )
                # phi(x) = min(exp(x), 1 + relu(x)), exact
                ek = ipool.tile([P, TC], F16, tag="ek")
                nc.scalar.activation(out=ek[:], in_=psk[:], func=AF.Exp,
                                     bias=bk_sb[:, lt:lt + 1])
                uk = ipool.tile([P, TC], F16, tag="uk")
                nc.vector.tensor_scalar(
                    out=uk[:], in0=psk[:], scalar1=bk1_sb[:, lt:lt + 1],
                    scalar2=1.0, op0=M.add, op1=M.max)
                pk = ipool.tile([P, TC], F16, tag="pk")
                nc.vector.tensor_tensor(out=pk[:], in0=uk[:], in1=ek[:], op=M.min)
                pk_t.append(pk)
                w = ipool.tile([P, TC], F16, tag="w")
                eng_wg.tensor_tensor(out=w[:], in0=pk[:], in1=bb_c[:, lt, :],
                                     op=M.mult)
                w_t.append(w)

            # ---- previous chunk's O-projection (fills the PE while this
            # chunk's elementwise chain completes; placed after the k phase
            # so its PSUM->SBUF copies clear the ACT queue before eq) ----
            if PIPE_O and c > 0:
                emit_O(c - 1, y_prev)

            # ---- v projections + scan ----
            s_new_t = []
            for lt in range(JT):
                jsl = ts(lt, P)
                psv = pproj.tile([P, TC], F32, tag="proj")
                for d in range(DT):
                    nc.tensor.matmul(
                        out=psv[:], lhsT=wsrc(wv_p, d, jsl),
                        rhs=xsrc(xv_c if c else None, xv0_p, d),
                        start=(d == 0), stop=(d == DT - 1) and not with_vbias,
                    )
                if with_vbias:
                    nc.tensor.matmul(out=psv[:], lhsT=bvr_sb[:, jsl],
                                     rhs=ones_sb[:], start=False, stop=True)
                pk, w = pk_t[lt], w_t[lt]
                g = ipool.tile([P, TC], F16, tag="g")
                eng_wg.tensor_tensor(out=g[:], in0=pk[:], in1=w[:], op=M.mult)
                a = ipool.tile([P, TC], F16, tag="a")
                nc.vector.tensor_scalar(out=a[:], in0=g[:], scalar1=-1.0,
                                        scalar2=1.0, op0=M.mult, op1=M.add)
                cc = ipool.tile([P, TC], F16, tag="cc")
                nc.vector.tensor_tensor(out=cc[:], in0=psv[:], in1=w[:], op=M.mult)
                s_new = spool.tile([P, TC], F16, tag=f"s{lt}")
                init = 0.0 if c == 0 else s_prev[lt][:, TC - 1:TC]
                nc.vector.tensor_tensor_scan(
                    out=s_new[:], data0=a[:], data1=cc[:], initial=init,
                    op0=M.mult, op1=M.add,
                )
                s_prev[lt] = s_new
                s_new_t.append(s_new)

            # ---- q projections + phi(q) + y = s * pq ----
            y_new = []
            for lt in range(JT):
                jsl = ts(lt, P)
                psq = pproj.tile([P, TC], F32, tag="proj")
                for d in range(DT):
                    nc.tensor.matmul(
                        out=psq[:], lhsT=wsrc(wq_p, d, jsl),
                        rhs=xsrc(xq_c if c else None, xq0_p, d),
                        start=(d == 0), stop=(d == DT - 1),
                    )
                eq = ipool.tile([P, TC], F16, tag="ek")
                nc.scalar.activation(out=eq[:], in_=psq[:], func=AF.Exp,
                                     bias=bq_sb[:, lt:lt + 1])
                uq = ipool.tile([P, TC], F16, tag="uk")
                nc.vector.tensor_scalar(
                    out=uq[:], in0=psq[:], scalar1=bq1_sb[:, lt:lt + 1],
                    scalar2=1.0, op0=M.add, op1=M.max)
                pq = ipool.tile([P, TC], F16, tag="pk")
                nc.vector.tensor_tensor(out=pq[:], in0=uq[:], in1=eq[:], op=M.min)
                y = spool.tile([P, TC], F16, tag=f"y{lt}")
                nc.vector.tensor_tensor(out=y[:], in0=s_new_t[lt][:], in1=pq[:],
                                        op=M.mult)
                y_new.append(y)
            y_prev = y_new

            if not PIPE_O:
                emit_O(c, y_prev)

        if PIPE_O:
            emit_O(NCH - 1, y_prev, final=True)

    nc.compile()
    return nc


_NC_CACHE = {}


def _get_nc(with_vbias):
    key = (with_vbias, GP_WG, PIPE_O, N_WARM, PP_BUFS, PO_BUFS)
    if key not in _NC_CACHE:
        _NC_CACHE[key] = build_nc(with_vbias)
    return _NC_CACHE[key]


def make_in_maps(query, key, value, beta, Wq, bq, Wk, bk, Wv, bv, Wb, bb, Wo, bo,
                 with_vbias):
    """Host-side shard prep: core_id = b*2 + hg."""

    def xpack(x):  # [S, H_DIM] -> [p, dt, s] fp16
        a = np.asarray(x, np.float32).T.reshape(DT, P, S)
        return np.ascontiguousarray(a.transpose(1, 0, 2)).astype(np.float16)

    def t16(x):
        return np.ascontiguousarray(np.asarray(x, np.float32).T).astype(np.float16)

    xqs = [xpack(query[b]) for b in range(B)]
    xks = [xpack(key[b]) for b in range(B)]
    xvs = [xpack(value[b]) for b in range(B)]
    # gate b computed host-side (0.4% of FLOPs), pre-broadcast per lane
    Wbf = np.asarray(Wb, np.float32)
    bbf0 = np.asarray(bb, np.float32)
    z = np.einsum('bsd,hd->bsh', np.asarray(beta, np.float32), Wbf) + bbf0
    bgate = 1.0 / (1.0 + np.exp(-z))                      # [B, S, 16]

    bqf = np.asarray(bq, np.float32)
    bkf = np.asarray(bk, np.float32)
    bvf = np.asarray(bv, np.float32)

    in_maps = []
    for b in range(B):
        for hg in range(HG):
            jsl = slice(hg * J, (hg + 1) * J)
            hsl = slice(hg * HPC, (hg + 1) * HPC)

            def lanes(v):  # [J] -> [128, 4] per lane-tile columns
                return np.ascontiguousarray(v[jsl].reshape(JT, P).T)

            # [S, 512] lane-broadcast gate -> [p, lt, s]
            rep = np.repeat(bgate[b][:, hsl], HEAD_DIM, axis=1).T  # [512, S]
            bl = np.ascontiguousarray(
                rep.reshape(JT, P, S).transpose(1, 0, 2)).astype(np.float16)

            m = {
                "xq": xqs[b], "xk": xks[b], "xv": xvs[b], "bbb": bl,
                "wq": t16(Wq[jsl]), "wk": t16(Wk[jsl]), "wv": t16(Wv[jsl]),
                "wo": t16(Wo[:, jsl]),
                "bq": lanes(bqf), "bk": lanes(bkf),
                "bq1": lanes(bqf) + 1.0, "bk1": lanes(bkf) + 1.0,
            }
            if with_vbias:
                m["bvr"] = bvf[jsl].reshape(1, J).astype(np.float16)
            in_maps.append(m)
    return in_maps


LAST_RESULTS = None


def kernel(**inputs):
    global LAST_RESULTS
    with_vbias = bool(np.any(np.asarray(inputs["bv"], np.float32)))
    nc = _get_nc(with_vbias)
    in_maps = make_in_maps(**inputs, with_vbias=with_vbias)
    res = run_bass_kernel_spmd(nc, in_maps, core_ids=list(range(NCORES)),
                               trace=bool(os.environ.get("DELTA_TRACE")))
    LAST_RESULTS = res
    bo = np.asarray(inputs["bo"], np.float32)
    out = np.empty((B, S, H_DIM), np.float32)
    for b in range(B):
        m = (res.results[2 * b]["out"].astype(np.float32)
             + res.results[2 * b + 1]["out"].astype(np.float32))
        out[b] = m.T + bo
    return out


# revision 28
# speedup vs baseline: 1.0558x; 1.0159x over previous
"""DeltaRule (diagonal-state linear attention) Bass kernel for 8 TRN2 cores.

Problem: nn_DeltaRule_20194936225992
  B=4, S=2048, H_DIM=1024, N_HEADS=16, HEAD_DIM=64.
  q/k/v/b projections, phi = elu+1, per-(b,h,d) scalar linear recurrence
      s_t = (1 - b_t*pk_t^2) * s_{t-1} + b_t*v_t*pk_t ;  y_t = s_t * pq_t
  out = y @ Wo.T + bo

Sharding: core = (batch b, head-group hg) with hg covering 8 heads.
Each core computes its partial O-projection (contraction over its 512
lanes); host sums the two head-group partials per batch, transposes
[o,t] -> [t,o] and adds bo.

On-device layout: lanes (h*64+d) on partitions, time on free dim.  The
recurrence runs as a hardware `tensor_tensor_scan` per [128,TC] tile,
chained across time chunks via the last column of the previous s.

Engine plan (per lane-tile, per chunk):
  PE:  Wq/Wk/Wv projections (weights stationary, x.T moving), O-projection
       of the PREVIOUS chunk (software-pipelined between this chunk's k and
       v phases so the PE never waits on the elementwise chain), and a few
       K=1 warmup matmuls at program start so the PE p-state ramps to
       2.4GHz during the DMA-bound preamble.
  ACT: E = exp(x+b) for phi (one op per projection via the exact identity
       phi(x) = elu(x)+1 = min(exp(x), 1+relu(x))), PSUM->SBUF O copies
       with fp32->fp16 downcast.
  DVE: u = max(x+b+1, 1) [tensor_scalar, two-scalar form], pk = min(u, E),
       w = pk*b, g = pk*w, a = 1-g [4x tensor_scalar], cc = v*w, the
       hardware scan, q-side u/pq, y = s*pq.  fp16 intermediates hit the
       DVE 2x/4x fast modes (GpSimd tensor_tensor measured ~1.3us vs DVE
       ~0.33us for the same op, so everything stays on DVE).
  The sigmoid gate b is computed on the host (0.4% of total FLOPs) and
  DMA'd pre-broadcast per lane.

All tensors fp16 on device (8x more accurate than bf16 at identical PE/DMA
cost; rel err 1.3e-3 vs 5.8e-3) except PSUM accumulators, the fp32 scan
state (internal to the scan instruction), and the per-lane bias columns.
Chunk-0 weight/x DMAs are sliced into 2-dt pieces and interleaved in
consumption order so the first matmul issues at ~11.5us (7.2us of that is
fixed NEFF preamble); wo's DMA is deferred behind chunk-1's x prefetch to
keep it off the DMA-backlog critical path.  The v-projection bias matmul
is skipped when bv == 0 (it is for this problem's inputs).

Measured on 8 axon trn2 cores: 132.2us HW exec (baseline 158.6us), PE
merged-busy 84% of span, rel err 1.27e-3.  PE floor for this sharding is
512 matmuls x 215ns = 110us; remaining overhead is the fixed preamble/
epilogue (~13us) and the DMA-paced chunk 0.  fp8 DoubleRow (2x PE) was
evaluated and rejected: e4m3 quantization alone gives 3.75% GEMM error vs
the 2% end-to-end budget.
"""

import os
import sys

for _p in ("/opt/trn_rl_repo", os.path.expanduser("~/.axon_site/_ro/trn_rl_repo")):
    if os.path.isdir(_p) and _p not in sys.path:
        sys.path.insert(0, _p)

import numpy as np  # noqa: E402

import concourse.bass as bass  # noqa: E402
import concourse.tile as tile  # noqa: E402
from concourse import bacc, mybir  # noqa: E402
from concourse.bass import ts  # noqa: E402
from concourse.bass_utils import run_bass_kernel_spmd  # noqa: E402

# problem constants (hardcoded per task rules)
B, S, H_DIM, N_HEADS, HEAD_DIM = 4, 2048, 1024, 16, 64
P = 128
NCORES = 8
HG = 2                      # head groups
J = 512                     # lanes per core  (8 heads * 64)
JT = J // P                 # 4 j-tiles
DT = H_DIM // P             # 8 contraction tiles
HPC = N_HEADS // HG         # 8 heads per core
TC = 512                    # time chunk (max PE moving dim / PSUM bank)
NCH = S // TC
# chunk-0 DMA piece plan (dt_start, n_dt): sliced so the first matmul can
# issue after one piece instead of the whole 1MB tensor
PLAN = [(0, 2), (2, 2), (4, 2), (6, 2)]
NPC = len(PLAN)
DT2PIECE = {}
for _i, (_s, _n) in enumerate(PLAN):
    for _d in range(_s, _s + _n):
        DT2PIECE[_d] = (_i, _d - _s)

F32 = mybir.dt.float32
F16 = mybir.dt.float16
AF = mybir.ActivationFunctionType
M = mybir.AluOpType

# env knobs for experiments
GP_WG = os.environ.get("DELTA_GP_WG", "0") != "0"   # w,g on GpSimd (slower)
PIPE_O = os.environ.get("DELTA_PIPE", "1") != "0"   # software-pipeline O-proj
N_WARM = int(os.environ.get("DELTA_WARM", "8"))     # PE p-state warmup matmuls
PP_BUFS = int(os.environ.get("DELTA_PP", "5"))      # pproj PSUM banks
PO_BUFS = int(os.environ.get("DELTA_PO", "3"))      # O-proj PSUM banks


def build_nc(with_vbias):
    nc = bacc.Bacc(trn_type="TRN2", target_bir_lowering=False, debug=False)

    # per-core inputs; x tensors host-packed as [p, dt, s] fp16
    xq = nc.dram_tensor("xq", [P, DT, S], F16, kind="ExternalInput").ap()
    xk = nc.dram_tensor("xk", [P, DT, S], F16, kind="ExternalInput").ap()
    xv = nc.dram_tensor("xv", [P, DT, S], F16, kind="ExternalInput").ap()
    bbb = nc.dram_tensor("bbb", [P, JT, S], F16, kind="ExternalInput").ap()
    wq = nc.dram_tensor("wq", [H_DIM, J], F16, kind="ExternalInput").ap()
    wk = nc.dram_tensor("wk", [H_DIM, J], F16, kind="ExternalInput").ap()
    wv = nc.dram_tensor("wv", [H_DIM, J], F16, kind="ExternalInput").ap()
    wo = nc.dram_tensor("wo", [J, H_DIM], F16, kind="ExternalInput").ap()
    bq = nc.dram_tensor("bq", [P, JT], F32, kind="ExternalInput").ap()
    bk = nc.dram_tensor("bk", [P, JT], F32, kind="ExternalInput").ap()
    bq1 = nc.dram_tensor("bq1", [P, JT], F32, kind="ExternalInput").ap()
    bk1 = nc.dram_tensor("bk1", [P, JT], F32, kind="ExternalInput").ap()
    if with_vbias:
        bvr = nc.dram_tensor("bvr", [1, J], F16, kind="ExternalInput").ap()
    out = nc.dram_tensor("out", [H_DIM, S], F16, kind="ExternalOutput").ap()

    from contextlib import ExitStack

    with tile.TileContext(nc) as tcx, ExitStack() as ctx:
        wpool = ctx.enter_context(tcx.tile_pool(name="weights", bufs=1))
        c0pool = ctx.enter_context(tcx.tile_pool(name="c0x", bufs=1))
        xpool = ctx.enter_context(tcx.tile_pool(name="xin", bufs=2))
        ipool = ctx.enter_context(tcx.tile_pool(name="inter", bufs=3))
        spool = ctx.enter_context(tcx.tile_pool(name="scan", bufs=2))
        opool = ctx.enter_context(tcx.tile_pool(name="osb", bufs=4))
        pproj = ctx.enter_context(
            tcx.tile_pool(name="pproj", bufs=PP_BUFS, space="PSUM"))
        po = ctx.enter_context(tcx.tile_pool(name="po", bufs=PO_BUFS, space="PSUM"))

        # --- PE p-state warmup: K=1 matmuls on zeroed operands keep the PE
        # continuously busy through the DMA-bound preamble so the first real
        # matmul runs at the full 2.4GHz clock instead of ramping ---
        if N_WARM:
            wa = wpool.tile([1, P], F16, tag="warm_a")
            wb = wpool.tile([1, TC], F16, tag="warm_b")
            nc.vector.memset(wa[:], 0.0)
            nc.vector.memset(wb[:], 0.0)
            pwarm = po.tile([P, TC], F32, tag="po", name="pwarm")
            for _ in range(N_WARM):
                nc.tensor.matmul(out=pwarm[:], lhsT=wa[:], rhs=wb[:],
                                 start=True, stop=True)

        # --- persistent weights / constants (per-2dt pieces) ---
        def wpieces(tag):
            return [wpool.tile([P, n, J], F16, tag=f"{tag}{i}", name=f"{tag}{i}")
                    for i, (_, n) in enumerate(PLAN)]

        wk_p, wv_p, wq_p = wpieces("wk"), wpieces("wv"), wpieces("wq")
        wo_sb = wpool.tile([P, JT, H_DIM], F16, tag="wo")
        bq_sb = wpool.tile([P, JT], F32, tag="bq")
        bk_sb = wpool.tile([P, JT], F32, tag="bk")
        bq1_sb = wpool.tile([P, JT], F32, tag="bq1")
        bk1_sb = wpool.tile([P, JT], F32, tag="bk1")
        if with_vbias:
            bvr_sb = wpool.tile([1, J], F16, tag="bvr")
            ones_sb = wpool.tile([1, TC], F16, tag="ones")

        # chunk-0 x pieces (separate tiles so matmuls start per-piece)
        def xpieces(tag):
            return [c0pool.tile([P, n, TC], F16, tag=f"{tag}{i}", name=f"{tag}{i}")
                    for i, (_, n) in enumerate(PLAN)]

        xk0_p, xv0_p, xq0_p = xpieces("xk0"), xpieces("xv0"), xpieces("xq0")

        def wsrc(w_p, d, jsl):
            i, off = DT2PIECE[d]
            return w_p[i][:, off, jsl]

        # --- chunk-0 startup DMA stream: interleave weights with x pieces ---
        def wdma(w_p, wt, i):
            s, n = PLAN[i]
            nc.sync.dma_start(
                out=w_p[i][:],
                in_=wt.rearrange("(dt p) j -> p dt j", p=P)[:, s:s + n, :])

        def xdma(x_p, xt, i):
            s, n = PLAN[i]
            nc.sync.dma_start(out=x_p[i][:], in_=xt[:, s:s + n, 0:TC])

        for i in range(NPC):
            wdma(wk_p, wk, i)
            xdma(xk0_p, xk, i)
            if i == 0:
                nc.sync.dma_start(out=bk_sb[:], in_=bk)
                nc.sync.dma_start(out=bk1_sb[:], in_=bk1)
        for i in range(NPC):
            wdma(wv_p, wv, i)
            xdma(xv0_p, xv, i)
        bb_c = xpool.tile([P, JT, TC], F16, tag="bbb")
        nc.sync.dma_start(out=bb_c[:], in_=bbb[:, :, 0:TC])
        if with_vbias:
            nc.sync.dma_start(out=bvr_sb[:], in_=bvr)
            nc.vector.memset(ones_sb[:], 1.0)
        for i in range(NPC):
            wdma(wq_p, wq, i)
            xdma(xq0_p, xq, i)
        nc.sync.dma_start(out=bq_sb[:], in_=bq)
        nc.sync.dma_start(out=bq1_sb[:], in_=bq1)
        # wo is dispatched after chunk-1's x prefetch (first needed by the
        # pipelined O(0) midway through chunk 1) to keep it out of the
        # DMA-backlog critical path

        s_prev = [None] * JT    # last-chunk scan state tile per lane-tile
        y_prev = [None] * JT    # previous chunk's y tiles (for pipelined O)

        eng_wg = nc.gpsimd if GP_WG else nc.vector

        def emit_O(cp, y_tiles, final=False):
            """O-projection of chunk cp: out[o,t] += wo[j,o]*y[j,t].

            The final chunk's PSUM->SBUF copies go to DVE (idle in the tail,
            while the ACT queue still drains the q-phase Exp ops)."""
            for ot in range(DT):
                pso = po.tile([P, TC], F32, tag="po")
                for lt in range(JT):
                    nc.tensor.matmul(
                        out=pso[:], lhsT=wo_sb[:, lt, ts(ot, P)],
                        rhs=y_tiles[lt][:],
                        start=(lt == 0), stop=(lt == JT - 1),
                    )
                o_sb = opool.tile([P, TC], F16, tag="osb")
                if final and ot % 2:
                    # the final chunk's O is copy-paced: alternate queues so
                    # neither ACT nor DVE serializes the PSUM bank recycling
                    nc.vector.tensor_copy(out=o_sb[:], in_=pso[:])
                else:
                    nc.scalar.copy(out=o_sb[:], in_=pso[:])
                nc.sync.dma_start(out=out[ts(ot, P), ts(cp, TC)], in_=o_sb[:])

        for c in range(NCH):
            # --- stream x chunk (c>0); chunk 0 was sliced above ---
            if c > 0:
                xk_c = xpool.tile([P, DT, TC], F16, tag="xk")
                nc.sync.dma_start(out=xk_c[:], in_=xk[:, :, ts(c, TC)])
                xv_c = xpool.tile([P, DT, TC], F16, tag="xv")
                nc.sync.dma_start(out=xv_c[:], in_=xv[:, :, ts(c, TC)])
                bb_c = xpool.tile([P, JT, TC], F16, tag="bbb")
                nc.sync.dma_start(out=bb_c[:], in_=bbb[:, :, ts(c, TC)])
                xq_c = xpool.tile([P, DT, TC], F16, tag="xq")
                nc.sync.dma_start(out=xq_c[:], in_=xq[:, :, ts(c, TC)])
                if c == 1:
                    nc.sync.dma_start(
                        out=wo_sb[:],
                        in_=wo.rearrange("(jt p) o -> p jt o", p=P))

            def xsrc(whole, pieces, d):
                if c == 0:
                    i, off = DT2PIECE[d]
                    return pieces[i][:, off, :]
                return whole[:, d, :]

            # ---- k projections + phi(k) + scan coefficients ----
            pk_t, w_t = [], []
            for lt in range(JT):
                jsl = ts(lt, P)
                psk = pproj.tile([P, TC], F32, tag="proj")
                for d in range(DT):
                    nc.tensor.matmul(
                        out=psk[:], lhsT=wsrc(wk_p, d, jsl),
                        rhs=xsrc(xk_c if c else None, xk0_p, d),
                        start=(d == 0), stop=(d == DT - 1),
                    )
                # phi(x) = min(exp(x), 1 + relu(x)), exact
                ek = ipool.tile([P, TC], F16, tag="ek")
                nc.scalar.activation(out=ek[:], in_=psk[:], func=AF.Exp,
                                     bias=bk_sb[:, lt:lt + 1])
                uk = ipool.tile([P, TC], F16, tag="uk")
                nc.vector.tensor_scalar(
                    out=uk[:], in0=psk[:], scalar1=bk1_sb[:, lt:lt + 1],
                    scalar2=1.0, op0=M.add, op1=M.max)
                pk = ipool.tile([P, TC], F16, tag="pk")
                nc.vector.tensor_tensor(out=pk[:], in0=uk[:], in1=ek[:], op=M.min)
                pk_t.append(pk)
                w = ipool.tile([P, TC], F16, tag="w")
                eng_wg.tensor_tensor(out=w[:], in0=pk[:], in1=bb_c[:, lt, :],
                                     op=M.mult)
                w_t.append(w)

            # ---- previous chunk's O-projection (fills the PE while this
            # chunk's elementwise chain completes; placed after the k phase
            # so its PSUM->SBUF copies clear the ACT queue before eq) ----
            if PIPE_O and c > 0:
                emit_O(c - 1, y_prev)

            # ---- v projections + scan ----
            s_new_t = []
            for lt in range(JT):
                jsl = ts(lt, P)
                psv = pproj.tile([P, TC], F32, tag="proj")
                for d in range(DT):
                    nc.tensor.matmul(
                        out=psv[:], lhsT=wsrc(wv_p, d, jsl),
                        rhs=xsrc(xv_c if c else None, xv0_p, d),
                        start=(d == 0), stop=(d == DT - 1) and not with_vbias,
                    )
                if with_vbias:
                    nc.tensor.matmul(out=psv[:], lhsT=bvr_sb[:, jsl],
                                     rhs=ones_sb[:], start=False, stop=True)
                pk, w = pk_t[lt], w_t[lt]
                g = ipool.tile([P, TC], F16, tag="g")
                eng_wg.tensor_tensor(out=g[:], in0=pk[:], in1=w[:], op=M.mult)
                a = ipool.tile([P, TC], F16, tag="a")
                nc.vector.tensor_scalar(out=a[:], in0=g[:], scalar1=-1.0,
                                        scalar2=1.0, op0=M.mult, op1=M.add)
                cc = ipool.tile([P, TC], F16, tag="cc")
                nc.vector.tensor_tensor(out=cc[:], in0=psv[:], in1=w[:], op=M.mult)
                s_new = spool.tile([P, TC], F16, tag=f"s{lt}")
                init = 0.0 if c == 0 else s_prev[lt][:, TC - 1:TC]
                nc.vector.tensor_tensor_scan(
                    out=s_new[:], data0=a[:], data1=cc[:], initial=init,
                    op0=M.mult, op1=M.add,
                )
                s_prev[lt] = s_new
                s_new_t.append(s_new)

            # ---- q projections + phi(q) + y = s * pq ----
            y_new = []
            for lt in range(JT):
                jsl = ts(lt, P)
                psq = pproj.tile([P, TC], F32, tag="proj")
                for d in range(DT):
                    nc.tensor.matmul(
                        out=psq[:], lhsT=wsrc(wq_p, d, jsl),
                        rhs=xsrc(xq_c if c else None, xq0_p, d),
                        start=(d == 0), stop=(d == DT - 1),
                    )
                eq = ipool.tile([P, TC], F16, tag="ek")
                nc.scalar.activation(out=eq[:], in_=psq[:], func=AF.Exp,
                                     bias=bq_sb[:, lt:lt + 1])
                uq = ipool.tile([P, TC], F16, tag="uk")
                nc.vector.tensor_scalar(
                    out=uq[:], in0=psq[:], scalar1=bq1_sb[:, lt:lt + 1],
                    scalar2=1.0, op0=M.add, op1=M.max)
                pq = ipool.tile([P, TC], F16, tag="pk")
                nc.vector.tensor_tensor(out=pq[:], in0=uq[:], in1=eq[:], op=M.min)
                y = spool.tile([P, TC], F16, tag=f"y{lt}")
                nc.vector.tensor_tensor(out=y[:], in0=s_new_t[lt][:], in1=pq[:],
                                        op=M.mult)
                y_new.append(y)
            y_prev = y_new

            if not PIPE_O:
                emit_O(c, y_prev)

        if PIPE_O:
            emit_O(NCH - 1, y_prev, final=True)

    nc.compile()
    return nc


_NC_CACHE = {}


def _get_nc(with_vbias):
    key = (with_vbias, GP_WG, PIPE_O, N_WARM, PP_BUFS, PO_BUFS)
    if key not in _NC_CACHE:
        _NC_CACHE[key] = build_nc(with_vbias)
    return _NC_CACHE[key]


def make_in_maps(query, key, value, beta, Wq, bq, Wk, bk, Wv, bv, Wb, bb, Wo, bo,
                 with_vbias):
    """Host-side shard prep: core_id = b*2 + hg."""

    def xpack(x):  # [S, H_DIM] -> [p, dt, s] fp16
        a = np.asarray(x, np.float32).T.reshape(DT, P, S)
        return np.ascontiguousarray(a.transpose(1, 0, 2)).astype(np.float16)

    def t16(x):
        return np.ascontiguousarray(np.asarray(x, np.float32).T).astype(np.float16)

    xqs = [xpack(query[b]) for b in range(B)]
    xks = [xpack(key[b]) for b in range(B)]
    xvs = [xpack(value[b]) for b in range(B)]
    # gate b computed host-side (0.4% of FLOPs), pre-broadcast per lane
    Wbf = np.asarray(Wb, np.float32)
    bbf0 = np.asarray(bb, np.float32)
    z = np.einsum('bsd,hd->bsh', np.asarray(beta, np.float32), Wbf) + bbf0
    bgate = 1.0 / (1.0 + np.exp(-z))                      # [B, S, 16]

    bqf = np.asarray(bq, np.float32)
    bkf = np.asarray(bk, np.float32)
    bvf = np.asarray(bv, np.float32)

    in_maps = []
    for b in range(B):
        for hg in range(HG):
            jsl = slice(hg * J, (hg + 1) * J)
            hsl = slice(hg * HPC, (hg + 1) * HPC)

            def lanes(v):  # [J] -> [128, 4] per lane-tile columns
                return np.ascontiguousarray(v[jsl].reshape(JT, P).T)

            # [S, 512] lane-broadcast gate -> [p, lt, s]
            rep = np.repeat(bgate[b][:, hsl], HEAD_DIM, axis=1).T  # [512, S]
            bl = np.ascontiguousarray(
                rep.reshape(JT, P, S).transpose(1, 0, 2)).astype(np.float16)

            m = {
                "xq": xqs[b], "xk": xks[b], "xv": xvs[b], "bbb": bl,
                "wq": t16(Wq[jsl]), "wk": t16(Wk[jsl]), "wv": t16(Wv[jsl]),
                "wo": t16(Wo[:, jsl]),
                "bq": lanes(bqf), "bk": lanes(bkf),
                "bq1": lanes(bqf) + 1.0, "bk1": lanes(bkf) + 1.0,
            }
            if with_vbias:
                m["bvr"] = bvf[jsl].reshape(1, J).astype(np.float16)
            in_maps.append(m)
    return in_maps


LAST_RESULTS = None


def kernel(**inputs):
    global LAST_RESULTS
    with_vbias = bool(np.any(np.asarray(inputs["bv"], np.float32)))
    nc = _get_nc(with_vbias)
    in_maps = make_in_maps(**inputs, with_vbias=with_vbias)
    res = run_bass_kernel_spmd(nc, in_maps, core_ids=list(range(NCORES)),
                               trace=bool(os.environ.get("DELTA_TRACE")))
    LAST_RESULTS = res
    bo = np.asarray(inputs["bo"], np.float32)
    out = np.empty((B, S, H_DIM), np.float32)
    for b in range(B):
        m = (res.results[2 * b]["out"].astype(np.float32)
             + res.results[2 * b + 1]["out"].astype(np.float32))
        out[b] = m.T + bo
    return out
